# revision 46
# baseline (speedup 1.0000x reference)
"""Multi-head self-attention with RoPE on 8 Trainium2 NeuronCores.

Sharding: data-parallel over batch (2) x tensor-parallel over heads (4 groups
of 4 heads). Each core computes its heads' attention plus a partial output
projection (row-sharded Wo); the host sums the 4 partials per batch.

Layout strategy (per core, no PE transposes anywhere):
  - x, Wq, Wk uploaded pre-transposed; Q/K projected directly into [e, s]
    ("transposed") layout. RoPE head-dims are host-permuted to [evens|odds]
    so the rotation is 2 elementwise muls + a 32-row-block swap (4 big
    SBUF->SBUF DMAs) + 1 add.
  - V projected in natural [s, e] layout with an all-ones column per head:
    the AV matmul then yields attn-out^T AND the softmax denominator.
  - Scores computed transposed (PT[k, q]) per 128-wide k-tile stripe; exp
    runs on the causal suffix only (scale 1/8 folded into ACT's affine);
    only the diagonal 128x128 block needs a mask multiply.
  - Softmax normalization per q-chunk as soon as its last k-tile lands.
"""

import os
import sys

import numpy as np

for _p in ("/opt/trn_rl_repo", "/root/.axon_site/_ro/trn_rl_repo"):
    if os.path.isdir(_p) and _p not in sys.path:
        sys.path.insert(0, _p)
        break

import concourse.bacc as bacc
import concourse.tile as tile
from concourse import mybir
from concourse.bass_utils import run_bass_kernel_spmd

B, S, D, H = 2, 2048, 1024, 16
DK = 64
THETA = 10000.0
NCORES = 8
HPC = H // (NCORES // B)  # heads per core = 4
E = HPC * DK              # local output dims per core = 256
ET = E // 128             # e-tiles per core = 2
F32 = mybir.dt.float32

# Compute dtype for matmuls: "f32r" (full-rate fp32 PE mode), "f32", "bf16".
COMPUTE = os.environ.get("BASS_ATTN_COMPUTE", "f32r")

_cache = {}


def _build_nc(repeat=1):
    nc = bacc.Bacc(
        "TRN2",
        target_bir_lowering=False,
        debug=False,
        enable_asserts=False,
        num_devices=NCORES,
    )
    # storage dtype for matmul-input tensors; float32r IS fp32 bits, but the
    # BIR verifier requires producers of fp32r-matmul inputs to declare it
    sdt = {"bf16": mybir.dt.bfloat16, "f32": F32, "f32r": mybir.dt.float32r}[COMPUTE]

    def mm(out, lhsT, rhs, **kw):
        nc.tensor.matmul(out, lhsT, rhs, **kw)

    # ---- I/O ----
    xT = nc.dram_tensor("xT", [D, S], sdt, kind="ExternalInput").ap()
    wqT = nc.dram_tensor("wqT", [D, E], sdt, kind="ExternalInput").ap()
    wkT = nc.dram_tensor("wkT", [D, E], sdt, kind="ExternalInput").ap()
    wvT = nc.dram_tensor("wvT", [D, E], sdt, kind="ExternalInput").ap()
    woT = nc.dram_tensor("woT", [E, D], sdt, kind="ExternalInput").ap()
    cosF = nc.dram_tensor("cosF", [128, S], F32, kind="ExternalInput").ap()
    sinF = nc.dram_tensor("sinF", [128, S], F32, kind="ExternalInput").ap()
    tri = nc.dram_tensor("tri", [128, 128], sdt, kind="ExternalInput").ap()
    ones4 = nc.dram_tensor("ones4", [128, 4], sdt, kind="ExternalInput").ap()
    out = nc.dram_tensor("out", [S, D], F32, kind="ExternalOutput").ap()

    DT8 = D // 128   # 8 d-tiles
    ST = S // 128    # 16 s-tiles
    QC = S // 512    # 4 q-chunks of 512

    with tile.TileContext(nc) as tc:
     for _rep in range(repeat):
      with (
          tc.tile_pool(name="const", bufs=1) as cp,
          tc.tile_pool(name="persist", bufs=1) as pp,
      ):
        # ---- constants / persistent SBUF ----
        cos_sb = cp.tile([128, S], F32, name="cos", tag="cos")
        sin_sb = cp.tile([128, S], F32, name="sin", tag="sin")
        tri_sb = cp.tile([128, 128], sdt, name="tri", tag="tri")
        qt_sb = [pp.tile([128, S], sdt, name=f"qt{e}", tag=f"qt{e}") for e in range(ET)]
        kt_sb = [pp.tile([128, S], sdt, name=f"kt{e}", tag=f"kt{e}") for e in range(ET)]
        v_sb = [pp.tile([128, HPC * 65], sdt, name=f"v{st}", tag=f"v{st}") for st in range(ST)]
        atn_sb = [pp.tile([128, S], sdt, name=f"atn{e}", tag=f"atn{e}") for e in range(ET)]
        wo_sb = [pp.tile([128, D], sdt, name=f"wo{e}", tag=f"wo{e}") for e in range(ET)]

        # ---- phase A: input loads + projections + RoPE ----
        with (
            tc.tile_pool(name="work", bufs=1) as ws,
            tc.tile_pool(name="psumA", bufs=1, space="PSUM") as psA,
        ):
            x_sb = [ws.tile([128, S], sdt, name=f"x{d}", tag=f"x{d}") for d in range(DT8)]
            w_sb = {}
            for nm in ("v", "q", "k"):
                w_sb[nm] = [ws.tile([128, E], sdt, name=f"w{nm}{d}", tag=f"w{nm}{d}")
                            for d in range(DT8)]
            # split loads over both HWDGE queues (SP + ACT; ACT idle here)
            for d in range(DT8):
                eng = nc.sync if d % 2 == 0 else nc.scalar
                other = nc.scalar if d % 2 == 0 else nc.sync
                other.dma_start(out=w_sb["v"][d], in_=wvT[d * 128:(d + 1) * 128, :])
                eng.dma_start(out=x_sb[d], in_=xT[d * 128:(d + 1) * 128, :])
                if d == 0:
                    nc.scalar.dma_start(out=cos_sb, in_=cosF)
                    nc.scalar.dma_start(out=sin_sb, in_=sinF)
            for d in range(DT8):
                (nc.sync if d % 2 else nc.scalar).dma_start(
                    out=w_sb["q"][d], in_=wqT[d * 128:(d + 1) * 128, :])
            for d in range(DT8):
                (nc.scalar if d % 2 else nc.sync).dma_start(
                    out=w_sb["k"][d], in_=wkT[d * 128:(d + 1) * 128, :])
            nc.scalar.dma_start(out=tri_sb, in_=tri)
            for e in range(ET):
                nc.scalar.dma_start(out=wo_sb[e], in_=woT[e * 128:(e + 1) * 128, :])

            # V projection (natural layout + ones column per head)
            for st in range(ST):
                nc.sync.dma_start(
                    out=v_sb[st].rearrange("p (h c) -> p h c", h=HPC)[:, :, 64:65],
                    in_=ones4.unsqueeze(-1))
                pv = psA.tile([128, E], F32, name="psv", tag="psv", bufs=3)
                for d in range(DT8):
                    mm(pv, lhsT=x_sb[d][:, st * 128:(st + 1) * 128],
                       rhs=w_sb["v"][d],
                       start=(d == 0), stop=(d == DT8 - 1))
                nc.vector.tensor_copy(
                    out=v_sb[st].rearrange("p (h c) -> p h c", h=HPC)[:, :, 0:64],
                    in_=pv.rearrange("p (h c) -> p h c", h=HPC))

            # Q/K projection + RoPE (rotate-half via 32-block DMA swap)
            def proj_rope(nm, dst, e):
                for sc in range(QC):
                    sl = slice(sc * 512, (sc + 1) * 512)
                    ps = psA.tile([128, 512], F32, name="psq", tag="psq", bufs=3)
                    for d in range(DT8):
                        mm(ps, lhsT=w_sb[nm][d][:, e * 128:(e + 1) * 128],
                           rhs=x_sb[d][:, sc * 512:(sc + 1) * 512],
                           start=(d == 0), stop=(d == DT8 - 1))
                    tsin = ws.tile([128, 512], F32, name="tsin", tag="tsin", bufs=3)
                    tsw = ws.tile([128, 512], F32, name="tsw", tag="tsw", bufs=3)
                    nc.vector.tensor_mul(out=dst[e][:, sl], in0=ps, in1=cos_sb[:, sl])
                    nc.vector.tensor_mul(out=tsin, in0=ps, in1=sin_sb[:, sl])
                    for blk in range(4):
                        (nc.sync if blk % 2 else nc.scalar).dma_start(
                            out=tsw[blk * 32:(blk + 1) * 32, :],
                            in_=tsin[(blk ^ 1) * 32:((blk ^ 1) + 1) * 32, :])
                    nc.vector.tensor_add(out=dst[e][:, sl], in0=dst[e][:, sl], in1=tsw)

            proj_rope("q", qt_sb, 0)
            proj_rope("k", kt_sb, 0)
            proj_rope("q", qt_sb, 1)
            proj_rope("k", kt_sb, 1)

        # ---- phase B: attention per head + inline output projection ----
        with (
            tc.tile_pool(name="attn", bufs=1) as asb,
            tc.tile_pool(name="psumB", bufs=1, space="PSUM") as psB,
        ):
            def head_attn(h, last=False):
                e = h // 2
                hb = (h % 2) * 64
                qrow = qt_sb[e][hb:hb + 64, :]
                krow = kt_sb[e][hb:hb + 64, :]
                avp = [psB.tile([65, 512], F32, name=f"avp{qc}", tag=f"avp{qc}",
                                bufs=1) for qc in range(QC)]

                def norm_qc(qc):
                    rs = asb.tile([128, 512], F32, name="rs", tag="rs", bufs=2)
                    r0 = asb.tile([1, 512], F32, name="r0", tag="r0", bufs=2)
                    rb = asb.tile([64, 512], F32, name="rb", tag="rb", bufs=2)
                    rr = asb.tile([64, 512], F32, name="rr", tag="rr", bufs=2)
                    nc.vector.tensor_copy(out=rs[64:65, :], in_=avp[qc][64:65, :])
                    nc.sync.dma_start(out=r0, in_=rs[64:65, :])
                    nc.gpsimd.partition_broadcast(rb, r0)
                    nc.vector.reciprocal(out=rr, in_=rb)
                    nc.vector.tensor_mul(
                        out=atn_sb[e][hb:hb + 64, qc * 512:(qc + 1) * 512],
                        in0=avp[qc][0:64, :], in1=rr)
                    if last:
                        # Wo for this q-chunk: all heads done with it; po
                        # spreads over the avp banks already released
                        for i_w, st in enumerate(range(qc * 4, qc * 4 + 4)):
                            for oc in range(2):
                                po = psB.tile([128, 512], F32, name="po",
                                              tag=f"avp{(i_w * 2 + oc) % (qc + 1)}",
                                              bufs=1)
                                for e2 in range(ET):
                                    mm(po,
                                       lhsT=atn_sb[e2][:, st * 128:(st + 1) * 128],
                                       rhs=wo_sb[e2][:, oc * 512:(oc + 1) * 512],
                                       start=(e2 == 0), stop=(e2 == ET - 1))
                                po_sb = asb.tile([128, 512], F32, name="po_sb",
                                                 tag="po_sb", bufs=4)
                                if qc == QC - 1:
                                    # tail batch: exp is done, ACT is idle
                                    nc.scalar.copy(out=po_sb, in_=po)
                                else:
                                    nc.vector.tensor_copy(out=po_sb, in_=po)
                                nc.sync.dma_start(
                                    out=out[st * 128:(st + 1) * 128,
                                            oc * 512:(oc + 1) * 512],
                                    in_=po_sb)

                for qh in range(2):
                    for kb in range(8 if qh == 0 else ST):
                        q0 = max(kb * 128, qh * 1024)
                        q1 = (qh + 1) * 1024
                        loc0 = q0 - qh * 1024
                        stripe = psB.tile([128, 1024], F32, name="stripe",
                                          tag="stripe", bufs=2)
                        pte = asb.tile([128, 1024], sdt, name="pte", tag="pte",
                                       bufs=4)
                        c0 = q0
                        while c0 < q1:
                            c1 = min(q1, (c0 // 512 + 1) * 512)
                            lsl = slice(c0 - qh * 1024, c1 - qh * 1024)
                            mm(stripe[:, lsl],
                               lhsT=krow[:, kb * 128:(kb + 1) * 128],
                               rhs=qrow[:, c0:c1], start=True, stop=True)
                            c0 = c1
                        nc.scalar.activation(
                            out=pte[:, loc0:1024], in_=stripe[:, loc0:1024],
                            func=mybir.ActivationFunctionType.Exp, scale=0.125)
                        if q0 == kb * 128:
                            nc.vector.tensor_mul(
                                out=pte[:, loc0:loc0 + 128],
                                in0=pte[:, loc0:loc0 + 128], in1=tri_sb)
                        c0 = q0
                        while c0 < q1:
                            c1 = min(q1, (c0 // 512 + 1) * 512)
                            qc = c0 // 512
                            asl = slice(c0 - qc * 512, c1 - qc * 512)
                            lsl = slice(c0 - qh * 1024, c1 - qh * 1024)
                            mm(avp[qc][:, asl],
                               lhsT=v_sb[kb][:, h * 65:h * 65 + 65],
                               rhs=pte[:, lsl],
                               start=(kb == 0),
                               stop=(kb == min(ST - 1, qc * 4 + 3)))
                            c0 = c1
                        if kb % 4 == 3 and kb // 4 in (2 * qh, 2 * qh + 1):
                            norm_qc(kb // 4)

            head_attn(0)
            head_attn(1)
            head_attn(2)
            head_attn(3, last=True)

    nc.compile()
    return nc


def _host_inputs(x, token_positions, Wq, Wk, Wv, Wo):
    sdt_np = np.float32
    if COMPUTE == "bf16":
        import ml_dtypes
        sdt_np = ml_dtypes.bfloat16
    perm = np.concatenate([np.arange(0, DK, 2), np.arange(1, DK, 2)])  # evens|odds
    inv_freq = THETA ** (-np.arange(0, DK, 2, dtype=np.float64) / DK)  # [32]
    in_maps = []
    for c in range(NCORES):
        b, g = divmod(c, NCORES // B)  # c // 4, c % 4
        heads = [(g * HPC + h) for h in range(HPC)]
        rows_rope = np.concatenate([h * DK + perm for h in heads])  # permuted
        rows_plain = np.concatenate([h * DK + np.arange(DK) for h in heads])
        pos = token_positions[b].astype(np.float64)  # [S]
        ang = inv_freq[:, None] * pos[None, :]  # [32, S]
        cos = np.cos(ang).astype(np.float32)
        sin = np.sin(ang).astype(np.float32)
        cosF = np.concatenate([cos, cos, cos, cos], axis=0)  # [128, S]
        sinF = np.concatenate([sin, -sin, sin, -sin], axis=0)
        tri = np.triu(np.ones((128, 128), np.float32))  # valid iff k<=q
        in_maps.append({
            "xT": np.ascontiguousarray(x[b].T).astype(sdt_np),
            "wqT": np.ascontiguousarray(Wq[rows_rope, :].T).astype(sdt_np),
            "wkT": np.ascontiguousarray(Wk[rows_rope, :].T).astype(sdt_np),
            "wvT": np.ascontiguousarray(Wv[rows_plain, :].T).astype(sdt_np),
            "woT": np.ascontiguousarray(Wo[:, rows_plain].T).astype(sdt_np),
            "cosF": cosF,
            "sinF": sinF,
            "tri": tri.astype(sdt_np),
            "ones4": np.ones((128, 4), sdt_np),
        })
    return in_maps


def kernel(x, token_positions, Wq, Wk, Wv, Wo, _debug=False):
    x = np.asarray(x, np.float32)
    token_positions = np.asarray(token_positions, np.int32)
    Wq, Wk, Wv, Wo = (np.asarray(w, np.float32) for w in (Wq, Wk, Wv, Wo))
    if "nc" not in _cache:
        _cache["nc"] = _build_nc()
    nc = _cache["nc"]
    in_maps = _host_inputs(x, token_positions, Wq, Wk, Wv, Wo)
    res = run_bass_kernel_spmd(
        nc, in_maps, core_ids=list(range(NCORES)), trace=False)
    outs = [r["out"] for r in res.results]
    full = np.zeros((B, S, D), np.float32)
    for c in range(NCORES):
        full[c // (NCORES // B)] += outs[c]
    if _debug:
        return full, res
    return full


# revision 54
# speedup vs baseline: 1.0262x; 1.0262x over previous
"""Multi-head self-attention with RoPE on 8 Trainium2 NeuronCores.

Sharding: data-parallel over batch (2) x tensor-parallel over heads (4 groups
of 4 heads). Each core computes its heads' attention plus a partial output
projection (row-sharded Wo); the host sums the 4 partials per batch.

Layout strategy (per core, no PE transposes anywhere):
  - x, Wq, Wk uploaded pre-transposed; Q/K projected directly into [e, s]
    ("transposed") layout. RoPE head-dims are host-permuted to [evens|odds]
    so the rotation is 2 elementwise muls + a 32-row-block swap (4 big
    SBUF->SBUF DMAs) + 1 add.
  - V projected in natural [s, e] layout with an all-ones column per head:
    the AV matmul then yields attn-out^T AND the softmax denominator.
  - Scores computed transposed (PT[k, q]) per 128-wide k-tile stripe; exp
    runs on the causal suffix only (scale 1/8 folded into ACT's affine);
    only the diagonal 128x128 block needs a mask multiply.
  - Softmax normalization per q-chunk as soon as its last k-tile lands.
"""

import os
import sys

import numpy as np

for _p in ("/opt/trn_rl_repo", "/root/.axon_site/_ro/trn_rl_repo"):
    if os.path.isdir(_p) and _p not in sys.path:
        sys.path.insert(0, _p)
        break

import concourse.bacc as bacc
import concourse.tile as tile
from concourse import mybir
from concourse.bass_utils import run_bass_kernel_spmd

B, S, D, H = 2, 2048, 1024, 16
DK = 64
THETA = 10000.0
NCORES = 8
HPC = H // (NCORES // B)  # heads per core = 4
E = HPC * DK              # local output dims per core = 256
ET = E // 128             # e-tiles per core = 2
F32 = mybir.dt.float32

# Compute dtype for matmuls: "f32r" (full-rate fp32 PE mode), "f32", "bf16".
COMPUTE = os.environ.get("BASS_ATTN_COMPUTE", "f32r")

_cache = {}


def _build_nc(repeat=1):
    nc = bacc.Bacc(
        "TRN2",
        target_bir_lowering=False,
        debug=False,
        enable_asserts=False,
        num_devices=NCORES,
    )
    # storage dtype for matmul-input tensors; float32r IS fp32 bits, but the
    # BIR verifier requires producers of fp32r-matmul inputs to declare it
    sdt = {"bf16": mybir.dt.bfloat16, "f32": F32, "f32r": mybir.dt.float32r}[COMPUTE]

    def mm(out, lhsT, rhs, **kw):
        nc.tensor.matmul(out, lhsT, rhs, **kw)

    # ---- I/O ----
    xT = nc.dram_tensor("xT", [D, S], sdt, kind="ExternalInput").ap()
    wqT = nc.dram_tensor("wqT", [D, E], sdt, kind="ExternalInput").ap()
    wkT = nc.dram_tensor("wkT", [D, E], sdt, kind="ExternalInput").ap()
    wvT = nc.dram_tensor("wvT", [D, E], sdt, kind="ExternalInput").ap()
    woT = nc.dram_tensor("woT", [E, D], sdt, kind="ExternalInput").ap()
    cosF = nc.dram_tensor("cosF", [128, S], F32, kind="ExternalInput").ap()
    sinF = nc.dram_tensor("sinF", [128, S], F32, kind="ExternalInput").ap()
    tri = nc.dram_tensor("tri", [128, 128], sdt, kind="ExternalInput").ap()
    ones4 = nc.dram_tensor("ones4", [128, 4], sdt, kind="ExternalInput").ap()
    out = nc.dram_tensor("out", [S, D], F32, kind="ExternalOutput").ap()

    DT8 = D // 128   # 8 d-tiles
    ST = S // 128    # 16 s-tiles
    QC = S // 512    # 4 q-chunks of 512

    with tile.TileContext(nc) as tc:
     for _rep in range(repeat):
      with (
          tc.tile_pool(name="const", bufs=1) as cp,
          tc.tile_pool(name="persist", bufs=1) as pp,
      ):
        # ---- constants / persistent SBUF ----
        cos_sb = cp.tile([128, S], F32, name="cos", tag="cos")
        sin_sb = cp.tile([128, S], F32, name="sin", tag="sin")
        tri_sb = cp.tile([128, 128], sdt, name="tri", tag="tri")
        qt_sb = [pp.tile([128, S], sdt, name=f"qt{e}", tag=f"qt{e}") for e in range(ET)]
        kt_sb = [pp.tile([128, S], sdt, name=f"kt{e}", tag=f"kt{e}") for e in range(ET)]
        v_sb = [pp.tile([128, HPC * 65], sdt, name=f"v{st}", tag=f"v{st}") for st in range(ST)]
        atn_sb = [pp.tile([128, S], sdt, name=f"atn{e}", tag=f"atn{e}") for e in range(ET)]
        wo_sb = [pp.tile([128, D], sdt, name=f"wo{e}", tag=f"wo{e}") for e in range(ET)]

        # ---- phase A: input loads + projections + RoPE ----
        with (
            tc.tile_pool(name="work", bufs=1) as ws,
            tc.tile_pool(name="psumA", bufs=1, space="PSUM") as psA,
        ):
            x_sb = [ws.tile([128, S], sdt, name=f"x{d}", tag=f"x{d}") for d in range(DT8)]
            w_sb = {}
            for nm in ("v", "q", "k"):
                w_sb[nm] = [ws.tile([128, E], sdt, name=f"w{nm}{d}", tag=f"w{nm}{d}")
                            for d in range(DT8)]
            # split loads over both HWDGE queues (SP + ACT; ACT idle here)
            for d in range(DT8):
                eng = nc.sync if d % 2 == 0 else nc.scalar
                other = nc.scalar if d % 2 == 0 else nc.sync
                other.dma_start(out=w_sb["v"][d], in_=wvT[d * 128:(d + 1) * 128, :])
                eng.dma_start(out=x_sb[d], in_=xT[d * 128:(d + 1) * 128, :])
                if d == 0:
                    nc.scalar.dma_start(out=cos_sb, in_=cosF)
                    nc.scalar.dma_start(out=sin_sb, in_=sinF)
            for d in range(DT8):
                (nc.sync if d % 2 else nc.scalar).dma_start(
                    out=w_sb["q"][d], in_=wqT[d * 128:(d + 1) * 128, :])
            for d in range(DT8):
                (nc.scalar if d % 2 else nc.sync).dma_start(
                    out=w_sb["k"][d], in_=wkT[d * 128:(d + 1) * 128, :])
            nc.scalar.dma_start(out=tri_sb, in_=tri)
            for e in range(ET):
                nc.scalar.dma_start(out=wo_sb[e], in_=woT[e * 128:(e + 1) * 128, :])

            # V projection (natural layout + ones column per head)
            for st in range(ST):
                nc.sync.dma_start(
                    out=v_sb[st].rearrange("p (h c) -> p h c", h=HPC)[:, :, 64:65],
                    in_=ones4.unsqueeze(-1))
                pv = psA.tile([128, E], F32, name="psv", tag="psv", bufs=4)
                for d in range(DT8):
                    mm(pv, lhsT=x_sb[d][:, st * 128:(st + 1) * 128],
                       rhs=w_sb["v"][d],
                       start=(d == 0), stop=(d == DT8 - 1))
                nc.vector.tensor_copy(
                    out=v_sb[st].rearrange("p (h c) -> p h c", h=HPC)[:, :, 0:64],
                    in_=pv.rearrange("p (h c) -> p h c", h=HPC))

            # Q/K projection + RoPE (rotate-half via 32-block DMA swap)
            def proj_rope(nm, dst, e):
                for sc in range(QC):
                    sl = slice(sc * 512, (sc + 1) * 512)
                    ps = psA.tile([128, 512], F32, name="psq", tag="psq", bufs=4)
                    for d in range(DT8):
                        mm(ps, lhsT=w_sb[nm][d][:, e * 128:(e + 1) * 128],
                           rhs=x_sb[d][:, sc * 512:(sc + 1) * 512],
                           start=(d == 0), stop=(d == DT8 - 1))
                    tsin = ws.tile([128, 512], F32, name="tsin", tag="tsin", bufs=5)
                    tsw = ws.tile([128, 512], F32, name="tsw", tag="tsw", bufs=5)
                    nc.vector.tensor_mul(out=dst[e][:, sl], in0=ps, in1=cos_sb[:, sl])
                    nc.vector.tensor_mul(out=tsin, in0=ps, in1=sin_sb[:, sl])
                    for blk in range(4):
                        (nc.sync if blk % 2 else nc.scalar).dma_start(
                            out=tsw[blk * 32:(blk + 1) * 32, :],
                            in_=tsin[(blk ^ 1) * 32:((blk ^ 1) + 1) * 32, :])
                    nc.vector.tensor_add(out=dst[e][:, sl], in0=dst[e][:, sl], in1=tsw)

            proj_rope("q", qt_sb, 0)
            proj_rope("k", kt_sb, 0)
            proj_rope("q", qt_sb, 1)
            proj_rope("k", kt_sb, 1)

        # ---- phase B: attention per head + inline output projection ----
        with (
            tc.tile_pool(name="attn", bufs=1) as asb,
            tc.tile_pool(name="psumB", bufs=1, space="PSUM") as psB,
        ):
            def head_attn(h, last=False):
                e = h // 2
                hb = (h % 2) * 64
                qrow = qt_sb[e][hb:hb + 64, :]
                krow = kt_sb[e][hb:hb + 64, :]
                avp = [psB.tile([65, 512], F32, name=f"avp{qc}", tag=f"avp{qc}",
                                bufs=1) for qc in range(QC)]

                def norm_qc(qc):
                    rs = asb.tile([128, 512], F32, name="rs", tag="rs", bufs=2)
                    r0 = asb.tile([1, 512], F32, name="r0", tag="r0", bufs=2)
                    rb = asb.tile([64, 512], F32, name="rb", tag="rb", bufs=2)
                    rr = asb.tile([64, 512], F32, name="rr", tag="rr", bufs=2)
                    nc.vector.tensor_copy(out=rs[64:65, :], in_=avp[qc][64:65, :])
                    nc.sync.dma_start(out=r0, in_=rs[64:65, :])
                    nc.gpsimd.partition_broadcast(rb, r0)
                    nc.vector.reciprocal(out=rr, in_=rb)
                    nc.vector.tensor_mul(
                        out=atn_sb[e][hb:hb + 64, qc * 512:(qc + 1) * 512],
                        in0=avp[qc][0:64, :], in1=rr)
                    if last:
                        # Wo for this q-chunk: all heads done with it; po
                        # spreads over the avp banks already released
                        for i_w, st in enumerate(range(qc * 4, qc * 4 + 4)):
                            for oc in range(2):
                                po = psB.tile([128, 512], F32, name="po",
                                              tag=f"avp{(i_w * 2 + oc) % (qc + 1)}",
                                              bufs=1)
                                for e2 in range(ET):
                                    mm(po,
                                       lhsT=atn_sb[e2][:, st * 128:(st + 1) * 128],
                                       rhs=wo_sb[e2][:, oc * 512:(oc + 1) * 512],
                                       start=(e2 == 0), stop=(e2 == ET - 1))
                                po_sb = asb.tile([128, 512], F32, name="po_sb",
                                                 tag="po_sb", bufs=4)
                                if qc == QC - 1:
                                    # tail batch: exp is done, ACT is idle
                                    nc.scalar.copy(out=po_sb, in_=po)
                                else:
                                    nc.vector.tensor_copy(out=po_sb, in_=po)
                                nc.sync.dma_start(
                                    out=out[st * 128:(st + 1) * 128,
                                            oc * 512:(oc + 1) * 512],
                                    in_=po_sb)

                for qh in range(2):
                    for kb in range(8 if qh == 0 else ST):
                        q0 = max(kb * 128, qh * 1024)
                        q1 = (qh + 1) * 1024
                        loc0 = q0 - qh * 1024
                        stripe = psB.tile([128, 1024], F32, name="stripe",
                                          tag="stripe", bufs=2)
                        pte = asb.tile([128, 1024], sdt, name="pte", tag="pte",
                                       bufs=6)
                        c0 = q0
                        while c0 < q1:
                            c1 = min(q1, (c0 // 512 + 1) * 512)
                            lsl = slice(c0 - qh * 1024, c1 - qh * 1024)
                            mm(stripe[:, lsl],
                               lhsT=krow[:, kb * 128:(kb + 1) * 128],
                               rhs=qrow[:, c0:c1], start=True, stop=True)
                            c0 = c1
                        nc.scalar.activation(
                            out=pte[:, loc0:1024], in_=stripe[:, loc0:1024],
                            func=mybir.ActivationFunctionType.Exp, scale=0.125)
                        if q0 == kb * 128:
                            nc.vector.tensor_mul(
                                out=pte[:, loc0:loc0 + 128],
                                in0=pte[:, loc0:loc0 + 128], in1=tri_sb)
                        c0 = q0
                        while c0 < q1:
                            c1 = min(q1, (c0 // 512 + 1) * 512)
                            qc = c0 // 512
                            asl = slice(c0 - qc * 512, c1 - qc * 512)
                            lsl = slice(c0 - qh * 1024, c1 - qh * 1024)
                            mm(avp[qc][:, asl],
                               lhsT=v_sb[kb][:, h * 65:h * 65 + 65],
                               rhs=pte[:, lsl],
                               start=(kb == 0),
                               stop=(kb == min(ST - 1, qc * 4 + 3)))
                            c0 = c1
                        if kb % 4 == 3 and kb // 4 in (2 * qh, 2 * qh + 1):
                            norm_qc(kb // 4)

            head_attn(0)
            head_attn(1)
            head_attn(2)
            head_attn(3, last=True)

    nc.compile()
    return nc


def _host_inputs(x, token_positions, Wq, Wk, Wv, Wo):
    sdt_np = np.float32
    if COMPUTE == "bf16":
        import ml_dtypes
        sdt_np = ml_dtypes.bfloat16
    perm = np.concatenate([np.arange(0, DK, 2), np.arange(1, DK, 2)])  # evens|odds
    inv_freq = THETA ** (-np.arange(0, DK, 2, dtype=np.float64) / DK)  # [32]
    in_maps = []
    for c in range(NCORES):
        b, g = divmod(c, NCORES // B)  # c // 4, c % 4
        heads = [(g * HPC + h) for h in range(HPC)]
        rows_rope = np.concatenate([h * DK + perm for h in heads])  # permuted
        rows_plain = np.concatenate([h * DK + np.arange(DK) for h in heads])
        pos = token_positions[b].astype(np.float64)  # [S]
        ang = inv_freq[:, None] * pos[None, :]  # [32, S]
        cos = np.cos(ang).astype(np.float32)
        sin = np.sin(ang).astype(np.float32)
        cosF = np.concatenate([cos, cos, cos, cos], axis=0)  # [128, S]
        sinF = np.concatenate([sin, -sin, sin, -sin], axis=0)
        tri = np.triu(np.ones((128, 128), np.float32))  # valid iff k<=q
        in_maps.append({
            "xT": np.ascontiguousarray(x[b].T).astype(sdt_np),
            "wqT": np.ascontiguousarray(Wq[rows_rope, :].T).astype(sdt_np),
            "wkT": np.ascontiguousarray(Wk[rows_rope, :].T).astype(sdt_np),
            "wvT": np.ascontiguousarray(Wv[rows_plain, :].T).astype(sdt_np),
            "woT": np.ascontiguousarray(Wo[:, rows_plain].T).astype(sdt_np),
            "cosF": cosF,
            "sinF": sinF,
            "tri": tri.astype(sdt_np),
            "ones4": np.ones((128, 4), sdt_np),
        })
    return in_maps


def kernel(x, token_positions, Wq, Wk, Wv, Wo, _debug=False):
    x = np.asarray(x, np.float32)
    token_positions = np.asarray(token_positions, np.int32)
    Wq, Wk, Wv, Wo = (np.asarray(w, np.float32) for w in (Wq, Wk, Wv, Wo))
    if "nc" not in _cache:
        _cache["nc"] = _build_nc()
    nc = _cache["nc"]
    in_maps = _host_inputs(x, token_positions, Wq, Wk, Wv, Wo)
    res = run_bass_kernel_spmd(
        nc, in_maps, core_ids=list(range(NCORES)), trace=False)
    outs = [r["out"] for r in res.results]
    full = np.zeros((B, S, D), np.float32)
    for c in range(NCORES):
        full[c // (NCORES // B)] += outs[c]
    if _debug:
        return full, res
    return full
